# revision 45
# baseline (speedup 1.0000x reference)
"""RNN-T joint network kernel for 8 Trainium2 NeuronCores.

out[b,t,u,c] = (enc[b,t,:] @ W[:, :D].T)[c] + (dec[b,u,:] @ W[:, D:].T)[c]

Sharding: data-parallel over (b, t-half): core i -> b = i//2, t-slab
[(i%2)*128, (i%2+1)*128).  Each core holds the full W, computes its
(128, 64, 1024) output slab (32 MB) and DMAs it out.  The output DMA
(32 MB/core at ~355-400 GB/s) is the roofline; everything else hides
under or ahead of it.

Host-side prep (part of the sharding/layout strategy): W, enc, dec are
passed pre-transposed so the contraction dim D sits on SBUF partitions
with no on-chip transposes.

Per-core dataflow:
  1. PE warm-up matmuls open the HAM clock gate during the input DMAs.
  2. GEMMs -> dec_proj (split from PSUM into exact bf16 hi+lo halves,
     error ~2^-18) and enc_proj (128,1024) fp32 in SBUF.
  3. For each u: two accumulating K=128 bf16 "selector" matmuls
     broadcast dec_proj[u,:] across all 128 partitions into PSUM.  The
     selector weight is column u of a 128x128 identity, free-dim
     broadcast to all 128 output columns (stride-0 AP) - no
     materialized selector tensor.  DVE adds enc_proj; groups of u's
     form contiguous 0.5-4 MB output DMAs (small first for an early
     first byte, large in steady state, small at the end for a short
     flush).
"""

import sys

import numpy as np

for _p in ("/opt/trn_rl_repo",):
    if _p not in sys.path:
        sys.path.insert(0, _p)

B, T, U, D, C = 4, 256, 64, 512, 1024
TSH = T // 2  # t-slab per core
NCORES = 8

_CACHE = {}


def _build_bass():
    import concourse.mybir as mybir
    from concourse import bacc
    from concourse.bass import ds
    from concourse.masks import make_identity
    from concourse.tile import TileContext

    f32 = mybir.dt.float32
    bf16 = mybir.dt.bfloat16
    add = mybir.AluOpType.add

    nc = bacc.Bacc("TRN2", target_bir_lowering=False, debug=False)
    # All inputs are pre-swizzled on the host into SBUF partition-major
    # layout (partition dim first, per-partition data contiguous in DRAM)
    # so input DMA descriptors are 8-16KB instead of 4KB.
    # wt k-index order interleaves enc/dec d-tiles: dt = [0,4,1,5,2,6,3,7].
    # s index: 0 = bf16 hi half, 1 = bf16 lo half (x = hi + lo exactly to
    # ~2^-18); projections run 3 bf16 passes (hi*hi + hi*lo + lo*hi).
    dect_d = nc.declare_dram_parameter("dect", [128, 2, 4, U], bf16, isOutput=False)
    wt_d = nc.declare_dram_parameter("wt", [128, 8, 2, C], bf16, isOutput=False)
    enct_d = nc.declare_dram_parameter("enct", [128, 2, 4, TSH], bf16, isOutput=False)
    o_d = nc.declare_dram_parameter("o", [TSH, U, C], f32, isOutput=True)
    ENCPOS = [0, 2, 4, 6]
    DECPOS = [1, 3, 5, 7]

    with TileContext(nc) as tc:
        with (
            tc.tile_pool(name="const", bufs=1) as cpool,
            tc.tile_pool(name="outp", bufs=3) as opool,
        ):
            # sel[k, u, m] = 1.0 if k == u else 0.0 (k on partitions; rows
            # U..127 all zero so the selector matmuls are K=128 full-array
            # ops).  Built on the otherwise-idle GpSimd.
            sel = cpool.tile([128, U, 128], bf16)
            nc.gpsimd.memset(sel[:], 0.0)
            nc.gpsimd.affine_select(
                out=sel[:],
                in_=sel[:],
                compare_op=mybir.AluOpType.not_equal,
                fill=1.0,
                base=0,
                pattern=[[-1, U], [0, 128]],
                channel_multiplier=1,
            )

            # ---- loads; small activation tiles first, then W in four 1MB
            # chunks (8KB descriptors, enc/dec d-tiles and hi/lo halves
            # interleaved) so both projection chains stay chunk-paced ----
            encT = cpool.tile([128, 2, 4, TSH], bf16)  # [p,s,j,t] = enc[t, j*128+p]
            nc.sync.dma_start(out=encT[:], in_=enct_d[:])
            decT = cpool.tile([128, 2, 4, U], bf16)  # [p,s,j,u] = dec[u, j*128+p]
            nc.sync.dma_start(out=decT[:], in_=dect_d[:])
            # wT[p, k, s, c] = W_s[c, dt(k)*128+p] with dt(k) = [0,4,1,5,2,6,3,7]
            wT = cpool.tile([128, 8, 2, 1024], bf16)
            for k in range(0, 8, 2):
                nc.sync.dma_start(out=wT[:, ds(k, 2)], in_=wt_d[:, ds(k, 2)])

            enc_proj = cpool.tile([TSH, C], f32)
            # dec_proj = dec_hi + dec_lo, both bf16 (exact split to ~2^-18);
            # rows U..127 zero so K=128 matmuls pick up nothing from them.
            dec_hi = cpool.tile([128, C], bf16)
            dec_lo = cpool.tile([128, C], bf16)
            nc.vector.memset(dec_hi[U:, :], 0.0)
            nc.vector.memset(dec_lo[U:, :], 0.0)

            # PE warm-up source tile (uninitialized; results discarded, so
            # garbage content is fine and the warm-up has no dependencies).
            warm_a = cpool.tile([128, 512], bf16)

            with tc.tile_pool(name="psS", bufs=2, space="PSUM") as ppool:
                # PE warm-up: dependency-free matmuls issued while the input
                # DMAs stream, so the HAM clock gate opens (1.2 -> 2.4 GHz)
                # before the projection matmuls run.  Results are discarded.
                # The dummy ScalarE copy pulls the one-time ACT_TABLE_LOAD
                # (~1.3us) off the enc_proj critical path.
                # Enough warm-up matmuls to bridge the PE-idle window until
                # the first W chunk lands (~12us); a >3.4us idle gap would
                # re-throttle the clock and double every projection matmul.
                wp = ppool.tile([128, 512], f32, tag="warm")
                for _ in range(20):
                    nc.tensor.matmul(
                        wp[:], warm_a[:, :128], warm_a[:], start=True, stop=True
                    )
                nc.scalar.copy(out=warm_a[:1, :32], in_=wp[:1, :32])

                # Four accumulation chains (enc/dec x c-half) interleaved
                # per dt, each on its own PSUM bank, so every W chunk is
                # consumed the moment it lands.
                ppe0 = ppool.tile([TSH, 512], f32, tag="proje")
                ppe1 = ppool.tile([TSH, 512], f32, tag="proje")
                ppd0 = ppool.tile([TSH, 512], f32, tag="projd")
                ppd1 = ppool.tile([TSH, 512], f32, tag="projd")
                ppe = [ppe0, ppe1]
                ppd = [ppd0, ppd1]
                # 3-pass hi/lo products: (sx, sw) in {(0,0), (0,1), (1,0)};
                # each (side, h) chain accumulates 12 bf16 matmuls.
                PASSES = [(0, 0), (0, 1), (1, 0)]
                for dt in range(4):
                    for h in range(2):
                        for pi, (sx, sw) in enumerate(PASSES):
                            nc.tensor.matmul(
                                ppe[h][:],
                                encT[:, sx, dt, :],
                                wT[:, ENCPOS[dt], sw, ds(h * 512, 512)],
                                start=(dt == 0 and pi == 0),
                                stop=(dt == 3 and pi == 2),
                            )
                    for h in range(2):
                        for pi, (sx, sw) in enumerate(PASSES):
                            nc.tensor.matmul(
                                ppd[h][:U],
                                decT[:, sx, dt, :],
                                wT[:, DECPOS[dt], sw, ds(h * 512, 512)],
                                start=(dt == 0 and pi == 0),
                                stop=(dt == 3 and pi == 2),
                            )
                # PSUM -> SBUF copies spread across ACT and DVE so they
                # don't serialize on one engine.
                nc.scalar.copy(out=enc_proj[:, ds(0, 512)], in_=ppe[0][:])
                nc.vector.tensor_copy(out=enc_proj[:, ds(512, 512)], in_=ppe[1][:])
                for h in range(2):
                    # hi/lo split straight from PSUM (no fp32 staging copy):
                    # ACT casts to bf16, DVE computes the bf16 remainder.
                    nc.scalar.copy(
                        out=dec_hi[:U, ds(h * 512, 512)], in_=ppd[h][:U]
                    )
                    nc.vector.tensor_tensor(
                        out=dec_lo[:U, ds(h * 512, 512)],
                        in0=ppd[h][:U],
                        in1=dec_hi[:U, ds(h * 512, 512)],
                        op=mybir.AluOpType.subtract,
                    )

            # ---- main loop over u ----
            # small at the start (early first output byte), big in the
            # middle (descriptor efficiency), small at the end (short flush)
            groups = [1, 1, 2, 4] + [8] * 5 + [4] * 3 + [2, 1, 1]
            assert sum(groups) == U
            with tc.tile_pool(name="psM", bufs=2, space="PSUM") as mpool:
                u0 = 0
                for gi, gsz in enumerate(groups):
                    ot = opool.tile([TSH, gsz, C], f32, tag="out")
                    if gi < 2:
                        # First two single-u groups: per-half ADDs and DMAs
                        # so the output stream's first bytes leave as soon
                        # as each c-half of the projections is ready.
                        pr = mpool.tile([TSH, 2, C], f32, tag="rep")
                        for h in range(2):
                            nc.tensor.matmul(
                                pr[:, 0, ds(h * 512, 512)],
                                sel[:, u0, :],
                                dec_hi[:, ds(h * 512, 512)],
                                start=True,
                                stop=False,
                            )
                            nc.tensor.matmul(
                                pr[:, 0, ds(h * 512, 512)],
                                sel[:, u0, :],
                                dec_lo[:, ds(h * 512, 512)],
                                start=False,
                                stop=True,
                            )
                            nc.vector.tensor_tensor(
                                out=ot[:, 0, ds(h * 512, 512)],
                                in0=pr[:, 0, ds(h * 512, 512)],
                                in1=enc_proj[:, ds(h * 512, 512)],
                                op=add,
                            )
                            nc.sync.dma_start(
                                out=o_d[:, ds(u0, 1), ds(h * 512, 512)],
                                in_=ot[:, :1, ds(h * 512, 512)],
                            )
                        u0 += gsz
                        continue
                    for jp in range((gsz + 1) // 2):
                        uw = min(2, gsz - jp * 2)  # u's in this psum tile
                        pr = mpool.tile([TSH, 2, C], f32, tag="rep")
                        for j2 in range(uw):
                            u = u0 + jp * 2 + j2
                            selw = sel[:, u, :]
                            for h in range(2):
                                nc.tensor.matmul(
                                    pr[:, j2, ds(h * 512, 512)],
                                    selw,
                                    dec_hi[:, ds(h * 512, 512)],
                                    start=True,
                                    stop=False,
                                )
                                nc.tensor.matmul(
                                    pr[:, j2, ds(h * 512, 512)],
                                    selw,
                                    dec_lo[:, ds(h * 512, 512)],
                                    start=False,
                                    stop=True,
                                )
                        nc.vector.tensor_tensor(
                            out=ot[:, ds(jp * 2, uw), :],
                            in0=pr[:, :uw, :],
                            in1=enc_proj[:, None, :].to_broadcast([TSH, uw, C]),
                            op=add,
                        )
                    # Alternate between the two HWDGE rings (SP and ACT) so
                    # output descriptor generation is not serialized on one
                    # ring.
                    eng = nc.sync if (u0 // 8) % 2 == 0 else nc.scalar
                    eng.dma_start(out=o_d[:, ds(u0, gsz), :], in_=ot[:, :gsz, :])
                    u0 += gsz

    nc.compile()
    return nc


def _get_nc():
    if "nc" not in _CACHE:
        _CACHE["nc"] = _build_bass()
    return _CACHE["nc"]


def _hilo(x):
    """float32 -> (2, ...) exact bf16 hi + lo halves (x ~= hi + lo)."""
    import ml_dtypes

    hi = x.astype(ml_dtypes.bfloat16)
    lo = (x - hi.astype(np.float32)).astype(ml_dtypes.bfloat16)
    return np.stack([hi, lo])


def _swizzle_act(xt):
    """(D, N) f32 -> (128, 2, D/128, N) bf16: partition-major, hi/lo split."""
    d, n = xt.shape
    s = _hilo(xt)  # (2, D, N)
    s = s.reshape(2, d // 128, 128, n)
    return np.ascontiguousarray(s.transpose(2, 0, 1, 3))


def _swizzle_w(wt, perm):
    """(2D, C) f32 -> (128, 8, 2, C) bf16: partition-major, d-tiles
    permuted, hi/lo split innermost."""
    dp, n = wt.shape
    s = _hilo(wt)  # (2, 2D, C)
    s = s.reshape(2, dp // 128, 128, n)[:, perm]  # (2, 8, 128, C)
    return np.ascontiguousarray(s.transpose(2, 1, 0, 3))


def _make_in_maps(encoder_outputs, decoder_outputs, W):
    enc = np.asarray(encoder_outputs, dtype=np.float32)
    dec = np.asarray(decoder_outputs, dtype=np.float32)
    w = np.asarray(W, dtype=np.float32)

    # (128, 8, 2, C) with d-tiles interleaved enc/dec: [0,4,1,5,2,6,3,7]
    wt = _swizzle_w(w.T, perm=[0, 4, 1, 5, 2, 6, 3, 7])

    in_maps = []
    for i in range(NCORES):
        b, th = i // 2, i % 2
        enct = _swizzle_act(enc[b, th * TSH : (th + 1) * TSH].T)  # (128,2,4,TSH)
        dect = _swizzle_act(dec[b].T)  # (128, 2, 4, U)
        in_maps.append({"enct": enct, "dect": dect, "wt": wt})
    return in_maps


def _run(encoder_outputs, decoder_outputs, W, trace=False):
    from concourse.bass_utils import run_bass_kernel_spmd

    nc = _get_nc()
    in_maps = _make_in_maps(encoder_outputs, decoder_outputs, W)
    res = run_bass_kernel_spmd(nc, in_maps, list(range(NCORES)), trace=trace)
    out = np.empty((B, T, U, C), dtype=np.float32)
    for i in range(NCORES):
        b, th = i // 2, i % 2
        out[b, th * TSH : (th + 1) * TSH] = res.results[i]["o"]
    return out, res


def kernel(encoder_outputs, decoder_outputs, W):
    out, _ = _run(encoder_outputs, decoder_outputs, W)
    return out


# revision 46
# speedup vs baseline: 1.0058x; 1.0058x over previous
"""RNN-T joint network kernel for 8 Trainium2 NeuronCores.

out[b,t,u,c] = (enc[b,t,:] @ W[:, :D].T)[c] + (dec[b,u,:] @ W[:, D:].T)[c]

Sharding: data-parallel over (b, t-half): core i -> b = i//2, t-slab
[(i%2)*128, (i%2+1)*128).  Each core holds the full W, computes its
(128, 64, 1024) output slab (32 MB) and DMAs it out.  The output DMA
(32 MB/core at ~355-400 GB/s) is the roofline; everything else hides
under or ahead of it.

Host-side prep (part of the sharding/layout strategy): W, enc, dec are
passed pre-transposed so the contraction dim D sits on SBUF partitions
with no on-chip transposes.

Per-core dataflow:
  1. PE warm-up matmuls open the HAM clock gate during the input DMAs.
  2. GEMMs -> dec_proj (split from PSUM into exact bf16 hi+lo halves,
     error ~2^-18) and enc_proj (128,1024) fp32 in SBUF.
  3. For each u: two accumulating K=128 bf16 "selector" matmuls
     broadcast dec_proj[u,:] across all 128 partitions into PSUM.  The
     selector weight is column u of a 128x128 identity, free-dim
     broadcast to all 128 output columns (stride-0 AP) - no
     materialized selector tensor.  DVE adds enc_proj; groups of u's
     form contiguous 0.5-4 MB output DMAs (small first for an early
     first byte, large in steady state, small at the end for a short
     flush).
"""

import sys

import numpy as np

for _p in ("/opt/trn_rl_repo",):
    if _p not in sys.path:
        sys.path.insert(0, _p)

B, T, U, D, C = 4, 256, 64, 512, 1024
TSH = T // 2  # t-slab per core
NCORES = 8

_CACHE = {}


def _build_bass():
    import concourse.mybir as mybir
    from concourse import bacc
    from concourse.bass import ds
    from concourse.masks import make_identity
    from concourse.tile import TileContext

    f32 = mybir.dt.float32
    bf16 = mybir.dt.bfloat16
    add = mybir.AluOpType.add

    nc = bacc.Bacc("TRN2", target_bir_lowering=False, debug=False)
    # All inputs are pre-swizzled on the host into SBUF partition-major
    # layout (partition dim first, per-partition data contiguous in DRAM)
    # so input DMA descriptors are 8-16KB instead of 4KB.
    # wt k-index order interleaves enc/dec d-tiles: dt = [0,4,1,5,2,6,3,7].
    # s index: 0 = bf16 hi half, 1 = bf16 lo half (x = hi + lo exactly to
    # ~2^-18); projections run 3 bf16 passes (hi*hi + hi*lo + lo*hi).
    dect_d = nc.declare_dram_parameter("dect", [128, 2, 4, U], bf16, isOutput=False)
    wt_d = nc.declare_dram_parameter("wt", [128, 8, 2, C], bf16, isOutput=False)
    enct_d = nc.declare_dram_parameter("enct", [128, 2, 4, TSH], bf16, isOutput=False)
    o_d = nc.declare_dram_parameter("o", [TSH, U, C], f32, isOutput=True)
    ENCPOS = [0, 2, 4, 6]
    DECPOS = [1, 3, 5, 7]

    with TileContext(nc) as tc:
        with (
            tc.tile_pool(name="const", bufs=1) as cpool,
            tc.tile_pool(name="outp", bufs=3) as opool,
        ):
            # sel[k, u, m] = 1.0 if k == u else 0.0 (k on partitions; rows
            # U..127 all zero so the selector matmuls are K=128 full-array
            # ops).  Built on the otherwise-idle GpSimd.
            sel = cpool.tile([128, U, 128], bf16)
            nc.gpsimd.memset(sel[:], 0.0)
            nc.gpsimd.affine_select(
                out=sel[:],
                in_=sel[:],
                compare_op=mybir.AluOpType.not_equal,
                fill=1.0,
                base=0,
                pattern=[[-1, U], [0, 128]],
                channel_multiplier=1,
            )

            # ---- loads; small activation tiles first, then W in four 1MB
            # chunks (8KB descriptors, enc/dec d-tiles and hi/lo halves
            # interleaved) so both projection chains stay chunk-paced ----
            encT = cpool.tile([128, 2, 4, TSH], bf16)  # [p,s,j,t] = enc[t, j*128+p]
            nc.sync.dma_start(out=encT[:], in_=enct_d[:])
            decT = cpool.tile([128, 2, 4, U], bf16)  # [p,s,j,u] = dec[u, j*128+p]
            nc.sync.dma_start(out=decT[:], in_=dect_d[:])
            # wT[p, k, s, c] = W_s[c, dt(k)*128+p] with dt(k) = [0,4,1,5,2,6,3,7]
            wT = cpool.tile([128, 8, 2, 1024], bf16)
            for k in range(0, 8, 2):
                nc.sync.dma_start(out=wT[:, ds(k, 2)], in_=wt_d[:, ds(k, 2)])

            enc_proj = cpool.tile([TSH, C], f32)
            # dec_proj = dec_hi + dec_lo, both bf16 (exact split to ~2^-18);
            # rows U..127 zero so K=128 matmuls pick up nothing from them.
            dec_hi = cpool.tile([128, C], bf16)
            dec_lo = cpool.tile([128, C], bf16)
            nc.vector.memset(dec_hi[U:, :], 0.0)
            nc.vector.memset(dec_lo[U:, :], 0.0)

            # PE warm-up source tile (uninitialized; results discarded, so
            # garbage content is fine and the warm-up has no dependencies).
            warm_a = cpool.tile([128, 512], bf16)

            with tc.tile_pool(name="psS", bufs=2, space="PSUM") as ppool:
                # PE warm-up: dependency-free matmuls issued while the input
                # DMAs stream, so the HAM clock gate opens (1.2 -> 2.4 GHz)
                # before the projection matmuls run.  Results are discarded.
                # The dummy ScalarE copy pulls the one-time ACT_TABLE_LOAD
                # (~1.3us) off the enc_proj critical path.
                # Enough warm-up matmuls to bridge the PE-idle window until
                # the first W chunk lands (~12us); a >3.4us idle gap would
                # re-throttle the clock and double every projection matmul.
                wp = ppool.tile([128, 512], f32, tag="warm")
                for _ in range(20):
                    nc.tensor.matmul(
                        wp[:], warm_a[:, :128], warm_a[:], start=True, stop=True
                    )
                nc.scalar.copy(out=warm_a[:1, :32], in_=wp[:1, :32])

                # Four accumulation chains (enc/dec x c-half) interleaved
                # per dt, each on its own PSUM bank, so every W chunk is
                # consumed the moment it lands.
                ppe0 = ppool.tile([TSH, 512], f32, tag="proje")
                ppe1 = ppool.tile([TSH, 512], f32, tag="proje")
                ppd0 = ppool.tile([TSH, 512], f32, tag="projd")
                ppd1 = ppool.tile([TSH, 512], f32, tag="projd")
                ppe = [ppe0, ppe1]
                ppd = [ppd0, ppd1]
                # 3-pass hi/lo products: (sx, sw) in {(0,0), (0,1), (1,0)};
                # each (side, h) chain accumulates 12 bf16 matmuls.
                PASSES = [(0, 0), (0, 1), (1, 0)]
                for dt in range(4):
                    for h in range(2):
                        for pi, (sx, sw) in enumerate(PASSES):
                            nc.tensor.matmul(
                                ppe[h][:],
                                encT[:, sx, dt, :],
                                wT[:, ENCPOS[dt], sw, ds(h * 512, 512)],
                                start=(dt == 0 and pi == 0),
                                stop=(dt == 3 and pi == 2),
                            )
                    for h in range(2):
                        for pi, (sx, sw) in enumerate(PASSES):
                            nc.tensor.matmul(
                                ppd[h][:U],
                                decT[:, sx, dt, :],
                                wT[:, DECPOS[dt], sw, ds(h * 512, 512)],
                                start=(dt == 0 and pi == 0),
                                stop=(dt == 3 and pi == 2),
                            )
                # PSUM -> SBUF copies spread across ACT and DVE so they
                # don't serialize on one engine.
                nc.scalar.copy(out=enc_proj[:, ds(0, 512)], in_=ppe[0][:])
                nc.vector.tensor_copy(out=enc_proj[:, ds(512, 512)], in_=ppe[1][:])
                for h in range(2):
                    # hi/lo split straight from PSUM (no fp32 staging copy):
                    # ACT casts to bf16, DVE computes the bf16 remainder.
                    nc.scalar.copy(
                        out=dec_hi[:U, ds(h * 512, 512)], in_=ppd[h][:U]
                    )
                    nc.vector.tensor_tensor(
                        out=dec_lo[:U, ds(h * 512, 512)],
                        in0=ppd[h][:U],
                        in1=dec_hi[:U, ds(h * 512, 512)],
                        op=mybir.AluOpType.subtract,
                    )

            # ---- main loop over u ----
            # small at the start (early first output byte), big in the
            # middle (descriptor efficiency), small at the end (short flush)
            groups = [1, 1, 2, 4] + [8] * 5 + [4] * 3 + [2, 1, 1]
            assert sum(groups) == U
            with tc.tile_pool(name="psM", bufs=2, space="PSUM") as mpool:
                u0 = 0
                for gi, gsz in enumerate(groups):
                    ot = opool.tile([TSH, gsz, C], f32, tag="out")
                    for jp in range((gsz + 1) // 2):
                        uw = min(2, gsz - jp * 2)  # u's in this psum tile
                        pr = mpool.tile([TSH, 2, C], f32, tag="rep")
                        for j2 in range(uw):
                            u = u0 + jp * 2 + j2
                            selw = sel[:, u, :]
                            for h in range(2):
                                nc.tensor.matmul(
                                    pr[:, j2, ds(h * 512, 512)],
                                    selw,
                                    dec_hi[:, ds(h * 512, 512)],
                                    start=True,
                                    stop=False,
                                )
                                nc.tensor.matmul(
                                    pr[:, j2, ds(h * 512, 512)],
                                    selw,
                                    dec_lo[:, ds(h * 512, 512)],
                                    start=False,
                                    stop=True,
                                )
                        nc.vector.tensor_tensor(
                            out=ot[:, ds(jp * 2, uw), :],
                            in0=pr[:, :uw, :],
                            in1=enc_proj[:, None, :].to_broadcast([TSH, uw, C]),
                            op=add,
                        )
                    # Alternate between the two HWDGE rings (SP and ACT) so
                    # output descriptor generation is not serialized on one
                    # ring.
                    eng = nc.sync if (u0 // 8) % 2 == 0 else nc.scalar
                    eng.dma_start(out=o_d[:, ds(u0, gsz), :], in_=ot[:, :gsz, :])
                    u0 += gsz

    nc.compile()
    return nc


def _get_nc():
    if "nc" not in _CACHE:
        _CACHE["nc"] = _build_bass()
    return _CACHE["nc"]


def _hilo(x):
    """float32 -> (2, ...) exact bf16 hi + lo halves (x ~= hi + lo)."""
    import ml_dtypes

    hi = x.astype(ml_dtypes.bfloat16)
    lo = (x - hi.astype(np.float32)).astype(ml_dtypes.bfloat16)
    return np.stack([hi, lo])


def _swizzle_act(xt):
    """(D, N) f32 -> (128, 2, D/128, N) bf16: partition-major, hi/lo split."""
    d, n = xt.shape
    s = _hilo(xt)  # (2, D, N)
    s = s.reshape(2, d // 128, 128, n)
    return np.ascontiguousarray(s.transpose(2, 0, 1, 3))


def _swizzle_w(wt, perm):
    """(2D, C) f32 -> (128, 8, 2, C) bf16: partition-major, d-tiles
    permuted, hi/lo split innermost."""
    dp, n = wt.shape
    s = _hilo(wt)  # (2, 2D, C)
    s = s.reshape(2, dp // 128, 128, n)[:, perm]  # (2, 8, 128, C)
    return np.ascontiguousarray(s.transpose(2, 1, 0, 3))


def _make_in_maps(encoder_outputs, decoder_outputs, W):
    enc = np.asarray(encoder_outputs, dtype=np.float32)
    dec = np.asarray(decoder_outputs, dtype=np.float32)
    w = np.asarray(W, dtype=np.float32)

    # (128, 8, 2, C) with d-tiles interleaved enc/dec: [0,4,1,5,2,6,3,7]
    wt = _swizzle_w(w.T, perm=[0, 4, 1, 5, 2, 6, 3, 7])

    in_maps = []
    for i in range(NCORES):
        b, th = i // 2, i % 2
        enct = _swizzle_act(enc[b, th * TSH : (th + 1) * TSH].T)  # (128,2,4,TSH)
        dect = _swizzle_act(dec[b].T)  # (128, 2, 4, U)
        in_maps.append({"enct": enct, "dect": dect, "wt": wt})
    return in_maps


def _run(encoder_outputs, decoder_outputs, W, trace=False):
    from concourse.bass_utils import run_bass_kernel_spmd

    nc = _get_nc()
    in_maps = _make_in_maps(encoder_outputs, decoder_outputs, W)
    res = run_bass_kernel_spmd(nc, in_maps, list(range(NCORES)), trace=trace)
    out = np.empty((B, T, U, C), dtype=np.float32)
    for i in range(NCORES):
        b, th = i // 2, i % 2
        out[b, th * TSH : (th + 1) * TSH] = res.results[i]["o"]
    return out, res


def kernel(encoder_outputs, decoder_outputs, W):
    out, _ = _run(encoder_outputs, decoder_outputs, W)
    return out


# revision 48
# speedup vs baseline: 1.1163x; 1.1098x over previous
"""RNN-T joint network kernel for 8 Trainium2 NeuronCores.

out[b,t,u,c] = (enc[b,t,:] @ W[:, :D].T)[c] + (dec[b,u,:] @ W[:, D:].T)[c]

Sharding: data-parallel over (b, t-half): core i -> b = i//2, t-slab
[(i%2)*128, (i%2+1)*128).  Each core holds the full W, computes its
(128, 64, 1024) output slab (32 MB) and DMAs it out.  The output DMA
(32 MB/core at ~355-400 GB/s) is the roofline; everything else hides
under or ahead of it.

Host-side prep (part of the sharding/layout strategy): W, enc, dec are
passed pre-transposed so the contraction dim D sits on SBUF partitions
with no on-chip transposes.

Per-core dataflow:
  1. PE warm-up matmuls open the HAM clock gate during the input DMAs.
  2. GEMMs -> dec_proj (split from PSUM into exact bf16 hi+lo halves,
     error ~2^-18) and enc_proj (128,1024) fp32 in SBUF.
  3. For each u: two accumulating K=128 bf16 "selector" matmuls
     broadcast dec_proj[u,:] across all 128 partitions into PSUM.  The
     selector weight is column u of a 128x128 identity, free-dim
     broadcast to all 128 output columns (stride-0 AP) - no
     materialized selector tensor.  DVE adds enc_proj; groups of u's
     form contiguous 0.5-4 MB output DMAs (small first for an early
     first byte, large in steady state, small at the end for a short
     flush).
"""

import sys

import numpy as np

for _p in ("/opt/trn_rl_repo",):
    if _p not in sys.path:
        sys.path.insert(0, _p)

B, T, U, D, C = 4, 256, 64, 512, 1024
TSH = T // 2  # t-slab per core
NCORES = 8

_CACHE = {}


def _build_bass():
    import concourse.mybir as mybir
    from concourse import bacc
    from concourse.bass import ds
    from concourse.masks import make_identity
    from concourse.tile import TileContext

    f32 = mybir.dt.float32
    bf16 = mybir.dt.bfloat16
    add = mybir.AluOpType.add

    nc = bacc.Bacc("TRN2", target_bir_lowering=False, debug=False)
    # All inputs are pre-swizzled on the host into SBUF partition-major
    # layout (partition dim first, per-partition data contiguous in DRAM)
    # so input DMA descriptors are 8-16KB instead of 4KB.
    # wt k-index order interleaves enc/dec d-tiles: dt = [0,4,1,5,2,6,3,7].
    # s index: 0 = bf16 hi half, 1 = bf16 lo half (x = hi + lo exactly to
    # ~2^-18); projections run 3 bf16 passes (hi*hi + hi*lo + lo*hi).
    dect_d = nc.declare_dram_parameter("dect", [128, 2, 4, U], bf16, isOutput=False)
    wt_d = nc.declare_dram_parameter("wt", [128, 8, 2, C], bf16, isOutput=False)
    enct_d = nc.declare_dram_parameter("enct", [128, 2, 4, TSH], bf16, isOutput=False)
    o_d = nc.declare_dram_parameter("o", [TSH, U, C], f32, isOutput=True)
    ENCPOS = [0, 2, 4, 6]
    DECPOS = [1, 3, 5, 7]

    with TileContext(nc) as tc:
        with (
            tc.tile_pool(name="const", bufs=1) as cpool,
            tc.tile_pool(name="outp", bufs=3) as opool,
        ):
            # sel[k, u, m] = 1.0 if k == u else 0.0 (k on partitions; rows
            # U..127 all zero so the selector matmuls are K=128 full-array
            # ops).  Built on the otherwise-idle GpSimd.
            sel = cpool.tile([128, U, 128], bf16)
            nc.gpsimd.memset(sel[:], 0.0)
            nc.gpsimd.affine_select(
                out=sel[:],
                in_=sel[:],
                compare_op=mybir.AluOpType.not_equal,
                fill=1.0,
                base=0,
                pattern=[[-1, U], [0, 128]],
                channel_multiplier=1,
            )

            # ---- loads; small activation tiles first, then W in four 1MB
            # chunks (8KB descriptors, enc/dec d-tiles and hi/lo halves
            # interleaved) so both projection chains stay chunk-paced ----
            encT = cpool.tile([128, 2, 4, TSH], bf16)  # [p,s,j,t] = enc[t, j*128+p]
            nc.sync.dma_start(out=encT[:], in_=enct_d[:])
            decT = cpool.tile([128, 2, 4, U], bf16)  # [p,s,j,u] = dec[u, j*128+p]
            nc.sync.dma_start(out=decT[:], in_=dect_d[:])
            # wT[p, k, s, c] = W_s[c, dt(k)*128+p] with dt(k) = [0,4,1,5,2,6,3,7]
            wT = cpool.tile([128, 8, 2, 1024], bf16)
            for k in range(0, 8, 2):
                nc.sync.dma_start(out=wT[:, ds(k, 2)], in_=wt_d[:, ds(k, 2)])

            enc_proj = cpool.tile([TSH, C], f32)
            # dec_proj = dec_hi + dec_lo, both bf16 (exact split to ~2^-18);
            # rows U..127 zero so K=128 matmuls pick up nothing from them.
            dec_hi = cpool.tile([128, C], bf16)
            dec_lo = cpool.tile([128, C], bf16)
            nc.vector.memset(dec_hi[U:, :], 0.0)
            nc.vector.memset(dec_lo[U:, :], 0.0)

            # PE warm-up source tile (uninitialized; results discarded, so
            # garbage content is fine and the warm-up has no dependencies).
            warm_a = cpool.tile([128, 512], bf16)

            with tc.tile_pool(name="psS", bufs=2, space="PSUM") as ppool:
                # PE warm-up: dependency-free matmuls issued while the input
                # DMAs stream, so the HAM clock gate opens (1.2 -> 2.4 GHz)
                # before the projection matmuls run.  Results are discarded.
                # The dummy ScalarE copy pulls the one-time ACT_TABLE_LOAD
                # (~1.3us) off the enc_proj critical path.
                # Enough warm-up matmuls to bridge the PE-idle window until
                # the first W chunk lands (~12us); a >3.4us idle gap would
                # re-throttle the clock and double every projection matmul.
                wp = ppool.tile([128, 512], f32, tag="warm")
                for _ in range(20):
                    nc.tensor.matmul(
                        wp[:], warm_a[:, :128], warm_a[:], start=True, stop=True
                    )
                nc.scalar.copy(out=warm_a[:1, :32], in_=wp[:1, :32])

                # Four accumulation chains (enc/dec x c-half) interleaved
                # per dt, each on its own PSUM bank, so every W chunk is
                # consumed the moment it lands.
                ppe0 = ppool.tile([TSH, 512], f32, tag="proje")
                ppe1 = ppool.tile([TSH, 512], f32, tag="proje")
                ppd0 = ppool.tile([TSH, 512], f32, tag="projd")
                ppd1 = ppool.tile([TSH, 512], f32, tag="projd")
                ppe = [ppe0, ppe1]
                ppd = [ppd0, ppd1]
                # 3-pass hi/lo products: (sx, sw) in {(0,0), (0,1), (1,0)};
                # each (side, h) chain accumulates 12 bf16 matmuls.
                PASSES = [(0, 0), (0, 1), (1, 0)]
                for dt in range(4):
                    for h in range(2):
                        for pi, (sx, sw) in enumerate(PASSES):
                            nc.tensor.matmul(
                                ppe[h][:],
                                encT[:, sx, dt, :],
                                wT[:, ENCPOS[dt], sw, ds(h * 512, 512)],
                                start=(dt == 0 and pi == 0),
                                stop=(dt == 3 and pi == 2),
                            )
                    for h in range(2):
                        for pi, (sx, sw) in enumerate(PASSES):
                            nc.tensor.matmul(
                                ppd[h][:U],
                                decT[:, sx, dt, :],
                                wT[:, DECPOS[dt], sw, ds(h * 512, 512)],
                                start=(dt == 0 and pi == 0),
                                stop=(dt == 3 and pi == 2),
                            )
                for h in range(2):
                    nc.scalar.copy(
                        out=enc_proj[:, ds(h * 512, 512)], in_=ppe[h][:]
                    )
                for h in range(2):
                    # hi/lo split straight from PSUM (no fp32 staging copy):
                    # ACT casts to bf16, DVE computes the bf16 remainder.
                    nc.scalar.copy(
                        out=dec_hi[:U, ds(h * 512, 512)], in_=ppd[h][:U]
                    )
                    nc.vector.tensor_tensor(
                        out=dec_lo[:U, ds(h * 512, 512)],
                        in0=ppd[h][:U],
                        in1=dec_hi[:U, ds(h * 512, 512)],
                        op=mybir.AluOpType.subtract,
                    )

            # ---- main loop over u ----
            # small at the start (early first output byte), big in the
            # middle (descriptor efficiency), small at the end (short flush)
            groups = [1, 1, 2, 4] + [8] * 5 + [4] * 3 + [2, 1, 1]
            assert sum(groups) == U
            with tc.tile_pool(name="psM", bufs=2, space="PSUM") as mpool:
                u0 = 0
                for gi, gsz in enumerate(groups):
                    ot = opool.tile([TSH, gsz, C], f32, tag="out")
                    for jp in range((gsz + 1) // 2):
                        uw = min(2, gsz - jp * 2)  # u's in this psum tile
                        pr = mpool.tile([TSH, 2, C], f32, tag="rep")
                        for j2 in range(uw):
                            u = u0 + jp * 2 + j2
                            selw = sel[:, u, :]
                            for h in range(2):
                                nc.tensor.matmul(
                                    pr[:, j2, ds(h * 512, 512)],
                                    selw,
                                    dec_hi[:, ds(h * 512, 512)],
                                    start=True,
                                    stop=False,
                                )
                                nc.tensor.matmul(
                                    pr[:, j2, ds(h * 512, 512)],
                                    selw,
                                    dec_lo[:, ds(h * 512, 512)],
                                    start=False,
                                    stop=True,
                                )
                        nc.vector.tensor_tensor(
                            out=ot[:, ds(jp * 2, uw), :],
                            in0=pr[:, :uw, :],
                            in1=enc_proj[:, None, :].to_broadcast([TSH, uw, C]),
                            op=add,
                        )
                    nc.sync.dma_start(
                        out=o_d[:, ds(u0, gsz), :], in_=ot[:, :gsz, :]
                    )
                    u0 += gsz

    nc.compile()
    return nc


def _get_nc():
    if "nc" not in _CACHE:
        _CACHE["nc"] = _build_bass()
    return _CACHE["nc"]


def _hilo(x):
    """float32 -> (2, ...) exact bf16 hi + lo halves (x ~= hi + lo)."""
    import ml_dtypes

    hi = x.astype(ml_dtypes.bfloat16)
    lo = (x - hi.astype(np.float32)).astype(ml_dtypes.bfloat16)
    return np.stack([hi, lo])


def _swizzle_act(xt):
    """(D, N) f32 -> (128, 2, D/128, N) bf16: partition-major, hi/lo split."""
    d, n = xt.shape
    s = _hilo(xt)  # (2, D, N)
    s = s.reshape(2, d // 128, 128, n)
    return np.ascontiguousarray(s.transpose(2, 0, 1, 3))


def _swizzle_w(wt, perm):
    """(2D, C) f32 -> (128, 8, 2, C) bf16: partition-major, d-tiles
    permuted, hi/lo split innermost."""
    dp, n = wt.shape
    s = _hilo(wt)  # (2, 2D, C)
    s = s.reshape(2, dp // 128, 128, n)[:, perm]  # (2, 8, 128, C)
    return np.ascontiguousarray(s.transpose(2, 1, 0, 3))


def _make_in_maps(encoder_outputs, decoder_outputs, W):
    enc = np.asarray(encoder_outputs, dtype=np.float32)
    dec = np.asarray(decoder_outputs, dtype=np.float32)
    w = np.asarray(W, dtype=np.float32)

    # (128, 8, 2, C) with d-tiles interleaved enc/dec: [0,4,1,5,2,6,3,7]
    wt = _swizzle_w(w.T, perm=[0, 4, 1, 5, 2, 6, 3, 7])

    in_maps = []
    for i in range(NCORES):
        b, th = i // 2, i % 2
        enct = _swizzle_act(enc[b, th * TSH : (th + 1) * TSH].T)  # (128,2,4,TSH)
        dect = _swizzle_act(dec[b].T)  # (128, 2, 4, U)
        in_maps.append({"enct": enct, "dect": dect, "wt": wt})
    return in_maps


def _run(encoder_outputs, decoder_outputs, W, trace=False):
    from concourse.bass_utils import run_bass_kernel_spmd

    nc = _get_nc()
    in_maps = _make_in_maps(encoder_outputs, decoder_outputs, W)
    res = run_bass_kernel_spmd(nc, in_maps, list(range(NCORES)), trace=trace)
    out = np.empty((B, T, U, C), dtype=np.float32)
    for i in range(NCORES):
        b, th = i // 2, i % 2
        out[b, th * TSH : (th + 1) * TSH] = res.results[i]["o"]
    return out, res


def kernel(encoder_outputs, decoder_outputs, W):
    out, _ = _run(encoder_outputs, decoder_outputs, W)
    return out
